# revision 2
# baseline (speedup 1.0000x reference)
"""Causal multi-head attention (QKV-packed) on 8 Trainium2 NeuronCores.

Sharding: pure head-parallel. B*H = 32 (batch, head) pairs -> 4 per core,
zero inter-core communication. Per head, flash-style causal attention is
computed entirely in the "transposed" orientation so no on-device
transposes are needed:

  - Host pre-lays-out Q^T, K^T as bf16 [D=128, S] (D on partitions) and V
    as k-blocks [128, D]; scores are computed transposed S_T[k, q] =
    (K^T_j).T @ Q^T into fp32 PSUM "windows" of up to 1024 columns
    spanning 2 PSUM banks, so one ACT instruction exps a whole window
    (halves the ScalarE per-instruction overhead vs per-block exp).
  - The diagonal quad of each q-strip is split into 4 triangular 128-col
    segments (which share one contiguous [128,512] window region -> ONE
    DVE mask add of a precomputed TRI4 mask) plus 3 clean segments.
  - P_T = exp(scale * S_T + mask) lands in SBUF as bf16; O^T[d, q] +=
    V_j.T @ P_T and den[q] += ones.T @ P_T accumulate in PSUM per strip.
  - No normalization on device: O^T and den are DMA'd out and the host
    computes O^T / den (the on-device reciprocal was ~50us of DVE time).
  - All matmuls are bf16 (full-rate, FastWeightLoad hides LDWEIGHTS).
"""

import sys

if "/opt/trn_rl_repo" not in sys.path:
    sys.path.insert(0, "/opt/trn_rl_repo")

import numpy as np
import ml_dtypes

B, S, H, D = 2, 2048, 16, 128
NCORES = 8
HPC = (B * H) // NCORES  # heads per core = 4
QS = 512   # q-strip width (one PSUM bank of fp32 for O^T)
KB = 128   # k-block (partition dim)
WIN = 1024  # exp window columns (2 PSUM banks fp32)
NEG = -1.0e30
SCALE = 1.0 / float(np.sqrt(D))
NSTRIP = S // QS  # 4

_nc_cache = {}


def _strip_windows(s):
    """Windows for q-strip s. Each window is a list of segments
    (j, qlo, qlen, woff): score block j contributes strip-local q columns
    [qlo, qlo+qlen) placed at window columns [woff, woff+qlen).
    Returns list of (segments, used_cols, tri_cols) where tri_cols is the
    width of the leading region needing the TRI4 additive causal mask."""
    wins = []
    # full (non-diagonal) k-blocks, paired two per window
    full = list(range(4 * s))
    for p in range(0, len(full), 2):
        segs = [(full[p], 0, QS, 0)]
        used = QS
        if p + 1 < len(full):
            segs.append((full[p + 1], 0, QS, QS))
            used = 2 * QS
        wins.append((segs, used, 0))
    J = 4 * s
    # diagonal window A: four 128-col triangular segments (contiguous, so a
    # single [128,512] mask add covers them) + clean parts of t=0 and t=2
    segs_a = [
        (J + 0, 0, 128, 0),
        (J + 1, 128, 128, 128),
        (J + 2, 256, 128, 256),
        (J + 3, 384, 128, 384),
        (J + 0, 128, 384, 512),
        (J + 2, 384, 128, 896),
    ]
    wins.append((segs_a, 1024, 512))
    # diagonal window B: clean part of t=1
    wins.append(([(J + 1, 256, 256, 0)], 256, 0))
    return wins


def _build_nc():
    import concourse.bass as bass  # noqa: F401
    import concourse.mybir as mybir
    from concourse import bacc
    from concourse.tile import TileContext

    f32 = mybir.dt.float32
    bf16 = mybir.dt.bfloat16
    Exp = mybir.ActivationFunctionType.Exp

    nc = bacc.Bacc()
    # Packed input per head [128, 3*S] bf16:
    # cols [0,S) = Q^T, [S,2S) = K^T, [2S,3S) = V swizzled so column
    # block j holds the V k-block [128, D] (v[p, j*KB+d] = V[j*KB+p, d]).
    qkvT = nc.declare_dram_parameter("qkvT", [HPC, 128, 3 * S], bf16, isOutput=False)
    cst = nc.declare_dram_parameter("cst", [128, 512], f32, isOutput=False)
    oT = nc.declare_dram_parameter("oT", [HPC, 128, S], f32, isOutput=True)
    den = nc.declare_dram_parameter("den", [HPC, 1, S], f32, isOutput=True)

    with TileContext(nc) as tc:
        with (
            nc.allow_low_precision(reason="bf16 throughout; tol is 2e-2"),
            tc.tile_pool(name="cpool", bufs=1) as cpool,
            tc.tile_pool(name="qkpool", bufs=2) as qkpool,
            tc.tile_pool(name="ptpool", bufs=3) as ptpool,
            tc.tile_pool(name="obpool", bufs=2) as obpool,
            tc.tile_pool(name="dnpool", bufs=2) as dnpool,
            tc.tile_pool(name="psw", bufs=2, space="PSUM") as psw,
            tc.tile_pool(name="pso", bufs=2, space="PSUM") as pso,
            tc.tile_pool(name="psd", bufs=2, space="PSUM") as psd,
        ):
            tri4 = cpool.tile([128, 512], f32)
            nc.sync.dma_start(out=tri4[:], in_=cst[:])
            ones_col = cpool.tile([128, 1], bf16)
            nc.vector.memset(ones_col[:], 1.0)

            for h in range(HPC):
                qkv_sb = qkpool.tile([128, 3 * S], bf16, tag="qkv_sb")
                if h == 0:
                    # split the first head's load so the first matmuls can
                    # start early: K^T blocks 0-3, Q^T strip 0, V blocks 0-3
                    for c0, c1 in (
                        (S, S + 512),
                        (0, 512),
                        (2 * S, 2 * S + 512),
                        (512, S),
                        (S + 512, 2 * S),
                        (2 * S + 512, 3 * S),
                    ):
                        nc.sync.dma_start(
                            out=qkv_sb[:, c0:c1], in_=qkvT[h][:, c0:c1]
                        )
                else:
                    nc.sync.dma_start(out=qkv_sb[:], in_=qkvT[h])
                qt_sb = qkv_sb[:, 0:S]
                kt_sb = qkv_sb[:, S : 2 * S]
                v_sb = qkv_sb[:, 2 * S : 3 * S]

                den_sb = dnpool.tile([1, S], f32, tag="den_sb")
                for s in range(NSTRIP):
                    o_ps = pso.tile([128, QS], f32, tag="o_ps")
                    den_ps = psd.tile([1, QS], f32, tag="den_ps")
                    wins = _strip_windows(s)
                    nseg = sum(len(w[0]) for w in wins)
                    seg_i = 0
                    for segs, used, tri_cols in wins:
                        w = psw.tile([128, WIN], f32, tag="w")
                        for j, qlo, qlen, woff in segs:
                            nc.tensor.matmul(
                                w[:, woff : woff + qlen],
                                lhsT=kt_sb[:, KB * j : KB * (j + 1)],
                                rhs=qt_sb[:, QS * s + qlo : QS * s + qlo + qlen],
                                start=True,
                                stop=True,
                            )
                        if tri_cols:
                            nc.vector.tensor_add(
                                w[:, 0:tri_cols], w[:, 0:tri_cols], tri4[:, 0:tri_cols]
                            )
                        pt = ptpool.tile([128, WIN], bf16, tag="pt")
                        nc.scalar.activation(
                            pt[:, 0:used], w[:, 0:used], Exp, scale=SCALE
                        )
                        for j, qlo, qlen, woff in segs:
                            first, last = (seg_i == 0), (seg_i == nseg - 1)
                            nc.tensor.matmul(
                                o_ps[:, qlo : qlo + qlen],
                                lhsT=v_sb[:, KB * j : KB * (j + 1)],
                                rhs=pt[:, woff : woff + qlen],
                                start=first,
                                stop=last,
                            )
                            nc.tensor.matmul(
                                den_ps[:, qlo : qlo + qlen],
                                lhsT=ones_col,
                                rhs=pt[:, woff : woff + qlen],
                                start=first,
                                stop=last,
                            )
                            seg_i += 1
                    # strip epilogue: evacuate PSUM (no normalization here —
                    # the host divides by den)
                    nc.vector.tensor_copy(
                        den_sb[:, QS * s : QS * (s + 1)], den_ps[:]
                    )
                    o_sb = obpool.tile([128, QS], f32, tag="o_sb")
                    nc.vector.tensor_copy(o_sb[:], o_ps[:])
                    nc.sync.dma_start(
                        out=oT[h][:, QS * s : QS * (s + 1)], in_=o_sb[:]
                    )
                nc.sync.dma_start(out=den[h], in_=den_sb[:])
    nc.compile()
    return nc


def get_nc():
    if "nc" not in _nc_cache:
        _nc_cache["nc"] = _build_nc()
    return _nc_cache["nc"]


def _build_const():
    dk = np.arange(128)[:, None]
    c = np.arange(128)[None, :]
    tri = np.where(dk <= c, 0.0, NEG).astype(np.float32)
    return np.tile(tri, (1, 4))


def make_in_maps(qkv):
    qkv = np.asarray(qkv, dtype=np.float32)
    qkv_b = qkv.astype(ml_dtypes.bfloat16)
    cst = _build_const()
    in_maps = []
    for core in range(NCORES):
        qkvT = np.empty((HPC, 128, 3 * S), ml_dtypes.bfloat16)
        for i in range(HPC):
            bh = core * HPC + i
            b, h = bh // H, bh % H
            qkvT[i, :, 0:S] = qkv_b[b, :, 0, h, :].T
            qkvT[i, :, S : 2 * S] = qkv_b[b, :, 1, h, :].T
            qkvT[i, :, 2 * S : 3 * S] = (
                qkv_b[b, :, 2, h, :]
                .reshape(S // KB, KB, D)
                .transpose(1, 0, 2)
                .reshape(KB, S)
            )
        in_maps.append({"qkvT": qkvT, "cst": cst})
    return in_maps


def assemble_out(results):
    out = np.empty((B, S, H, D), np.float32)
    for core in range(NCORES):
        oTc = results[core]["oT"]  # [HPC, 128, S]
        dnc = results[core]["den"]  # [HPC, 1, S]
        for i in range(HPC):
            bh = core * HPC + i
            b, h = bh // H, bh % H
            out[b, :, h, :] = oTc[i].T / dnc[i][0][:, None]
    return out


def kernel(qkv):
    from concourse.bass_utils import run_bass_kernel_spmd

    in_maps = make_in_maps(qkv)
    nc = get_nc()
    res = run_bass_kernel_spmd(nc, in_maps, list(range(NCORES)))
    return assemble_out(res.results)


# revision 4
# speedup vs baseline: 1.4461x; 1.4461x over previous
"""Causal multi-head attention (QKV-packed) on 8 Trainium2 NeuronCores.

Sharding: pure head-parallel. B*H = 32 (batch, head) pairs -> 4 per core,
zero inter-core communication. Per head, flash-style causal attention is
computed entirely in the "transposed" orientation so no on-device
transposes are needed:

  - Host pre-lays-out Q^T, K^T as bf16 [D=128, S] (D on partitions) and V
    as k-blocks [128, D]; scores are computed transposed S_T[k, q] =
    (K^T_j).T @ Q^T into fp32 PSUM "windows" of up to 1024 columns
    spanning 2 PSUM banks, so one ACT instruction exps a whole window
    (halves the ScalarE per-instruction overhead vs per-block exp).
  - The diagonal quad of each q-strip is split into 4 triangular 128-col
    segments (which share one contiguous [128,512] window region -> ONE
    DVE mask add of a precomputed TRI4 mask and ONE merged den matmul)
    plus 3 clean segments.
  - P_T = exp(scale * S_T + mask) lands in SBUF as bf16; O^T[d, q] +=
    V_j.T @ P_T and den[q] += ones.T @ P_T accumulate in PSUM per strip.
  - The PE instruction stream is software-pipelined one window deep:
    window i's PV/den matmuls (which wait on exp_i) are emitted AFTER
    window i+1's score matmuls, so the strict-FIFO PE queue always has
    independent work while ScalarE runs exp and never goes idle (idle
    gaps > ~3.4us re-throttle the PE clock to 1.2 GHz via HAM).
  - No normalization on device: O^T and den are DMA'd out and the host
    computes O^T / den (the on-device reciprocal was ~50us of DVE time).
  - All matmuls are bf16 (full-rate, FastWeightLoad on 128-col weights).
"""

import sys

if "/opt/trn_rl_repo" not in sys.path:
    sys.path.insert(0, "/opt/trn_rl_repo")

import numpy as np
import ml_dtypes

B, S, H, D = 2, 2048, 16, 128
NCORES = 8
HPC = (B * H) // NCORES  # heads per core = 4
QS = 512   # q-strip width (one PSUM bank of fp32 for O^T)
KB = 128   # k-block (partition dim)
WIN = 1024  # exp window columns (2 PSUM banks fp32)
NEG = -1.0e30
SCALE = 1.0 / float(np.sqrt(D))
NSTRIP = S // QS  # 4

_nc_cache = {}


def _strip_windows(s):
    """Windows for q-strip s. Each window is
    (score_segs, den_segs, used_cols, tri_cols) with score segments
    (j, qlo, qlen, woff): block j contributes strip-local q columns
    [qlo, qlo+qlen) placed at window columns [woff, woff+qlen). tri_cols
    is the width of the leading region needing the TRI4 causal mask add.
    den_segs are (qlo, qlen, woff) only; the 4 tri segments merge into
    one den matmul since woff == qlo throughout [0, 512)."""
    wins = []
    full = list(range(4 * s))
    for p in range(0, len(full), 2):
        segs = [(full[p], 0, QS, 0)]
        used = QS
        if p + 1 < len(full):
            segs.append((full[p + 1], 0, QS, QS))
            used = 2 * QS
        dsegs = [(qlo, qlen, woff) for _, qlo, qlen, woff in segs]
        wins.append((segs, dsegs, used, 0))
    J = 4 * s
    # diagonal window A: four 128-col triangular segments packed at window
    # cols [0,512) with woff == qlo + clean parts of t=0 and t=2; same-lhsT
    # segments adjacent in emission order.
    segs_a = [
        (J + 0, 0, 128, 0),
        (J + 0, 128, 384, 512),
        (J + 1, 128, 128, 128),
        (J + 2, 256, 128, 256),
        (J + 2, 384, 128, 896),
        (J + 3, 384, 128, 384),
    ]
    dsegs_a = [(0, 512, 0), (128, 384, 512), (384, 128, 896)]
    wins.append((segs_a, dsegs_a, 1024, 512))
    # diagonal window B: clean part of t=1
    wins.append(([(J + 1, 256, 256, 0)], [(256, 256, 0)], 256, 0))
    return wins


def _build_nc():
    import concourse.bass as bass  # noqa: F401
    import concourse.mybir as mybir
    from concourse import bacc
    from concourse.tile import TileContext

    f32 = mybir.dt.float32
    bf16 = mybir.dt.bfloat16
    Exp = mybir.ActivationFunctionType.Exp

    nc = bacc.Bacc()
    # Packed input per head [128, 3*S] bf16:
    # cols [0,S) = Q^T, [S,2S) = K^T, [2S,3S) = V swizzled so column
    # block j holds the V k-block [128, D] (v[p, j*KB+d] = V[j*KB+p, d]).
    qkvT = nc.declare_dram_parameter("qkvT", [HPC, 128, 3 * S], bf16, isOutput=False)
    cst = nc.declare_dram_parameter("cst", [128, 512], f32, isOutput=False)
    oT = nc.declare_dram_parameter("oT", [HPC, 128, S], f32, isOutput=True)
    den = nc.declare_dram_parameter("den", [HPC, 1, S], f32, isOutput=True)

    # Flat job list: one entry per exp-window, in execution order.
    jobs = []
    for h in range(HPC):
        for s in range(NSTRIP):
            wins = _strip_windows(s)
            for wi, (segs, dsegs, used, tri_cols) in enumerate(wins):
                jobs.append(
                    dict(
                        h=h, s=s, segs=segs, dsegs=dsegs, used=used,
                        tri=tri_cols, first=(wi == 0), last=(wi == len(wins) - 1),
                    )
                )

    with TileContext(nc) as tc:
        with (
            nc.allow_low_precision(reason="bf16 throughout; tol is 2e-2"),
            tc.tile_pool(name="cpool", bufs=1) as cpool,
            tc.tile_pool(name="qkpool", bufs=2) as qkpool,
            tc.tile_pool(name="ptpool", bufs=3) as ptpool,
            tc.tile_pool(name="obpool", bufs=2) as obpool,
            tc.tile_pool(name="dnpool", bufs=2) as dnpool,
            tc.tile_pool(name="psw", bufs=2, space="PSUM") as psw,
            tc.tile_pool(name="pso", bufs=2, space="PSUM") as pso,
            tc.tile_pool(name="psd", bufs=2, space="PSUM") as psd,
        ):
            tri4 = cpool.tile([128, 512], f32)
            nc.sync.dma_start(out=tri4[:], in_=cst[:])
            ones_col = cpool.tile([128, 1], bf16)
            nc.vector.memset(ones_col[:], 1.0)

            heads = {}   # h -> (qt_sb, kt_sb, v_sb, den_sb)
            strips = {}  # live strip state: (h, s) -> (o_ps, den_ps)

            def load_head(h):
                qkv_sb = qkpool.tile([128, 3 * S], bf16, tag="qkv_sb")
                if h == 0:
                    # split the first head's load so the first matmuls can
                    # start early: K^T blocks 0-3, Q^T strip 0, V blocks 0-3
                    for c0, c1 in (
                        (S, S + 512),
                        (0, 512),
                        (2 * S, 2 * S + 512),
                        (512, S),
                        (S + 512, 2 * S),
                        (2 * S + 512, 3 * S),
                    ):
                        nc.sync.dma_start(out=qkv_sb[:, c0:c1], in_=qkvT[h][:, c0:c1])
                else:
                    nc.sync.dma_start(out=qkv_sb[:], in_=qkvT[h])
                den_sb = dnpool.tile([1, S], f32, tag="den_sb")
                heads[h] = (
                    qkv_sb[:, 0:S],
                    qkv_sb[:, S : 2 * S],
                    qkv_sb[:, 2 * S : 3 * S],
                    den_sb,
                )

            def emit_front(job):
                """Score matmuls + mask + exp for one window; returns the
                pt tile for the back half."""
                h, s = job["h"], job["s"]
                qt_sb, kt_sb, _, _ = heads[h]
                if job["first"]:
                    strips[(h, s)] = (
                        pso.tile([128, QS], f32, tag="o_ps", name="o_ps"),
                        psd.tile([1, QS], f32, tag="den_ps", name="den_ps"),
                    )
                w = psw.tile([128, WIN], f32, tag="w")
                for j, qlo, qlen, woff in job["segs"]:
                    nc.tensor.matmul(
                        w[:, woff : woff + qlen],
                        lhsT=kt_sb[:, KB * j : KB * (j + 1)],
                        rhs=qt_sb[:, QS * s + qlo : QS * s + qlo + qlen],
                        start=True,
                        stop=True,
                    )
                if job["tri"]:
                    nc.vector.tensor_add(
                        w[:, 0 : job["tri"]], w[:, 0 : job["tri"]],
                        tri4[:, 0 : job["tri"]],
                    )
                pt = ptpool.tile([128, WIN], bf16, tag="pt")
                nc.scalar.activation(
                    pt[:, 0 : job["used"]], w[:, 0 : job["used"]], Exp, scale=SCALE
                )
                return pt

            def emit_back(job, pt):
                """PV + den matmuls (waiting on exp) and, for the last
                window of a strip, the strip epilogue."""
                h, s = job["h"], job["s"]
                _, _, v_sb, den_sb = heads[h]
                o_ps, den_ps = strips[(h, s)]
                for j, qlo, qlen, woff in job["segs"]:
                    nc.tensor.matmul(
                        o_ps[:, qlo : qlo + qlen],
                        lhsT=v_sb[:, KB * j : KB * (j + 1)],
                        rhs=pt[:, woff : woff + qlen],
                        start=job["first"] and (j, qlo) == job["segs"][0][:2],
                        stop=job["last"] and (j, qlo) == job["segs"][-1][:2],
                    )
                for di, (qlo, qlen, woff) in enumerate(job["dsegs"]):
                    nc.tensor.matmul(
                        den_ps[:, qlo : qlo + qlen],
                        lhsT=ones_col,
                        rhs=pt[:, woff : woff + qlen],
                        start=job["first"] and di == 0,
                        stop=job["last"] and di == len(job["dsegs"]) - 1,
                    )
                if job["last"]:
                    nc.vector.tensor_copy(den_sb[:, QS * s : QS * (s + 1)], den_ps[:])
                    o_sb = obpool.tile([128, QS], f32, tag="o_sb")
                    nc.vector.tensor_copy(o_sb[:], o_ps[:])
                    nc.sync.dma_start(out=oT[h][:, QS * s : QS * (s + 1)], in_=o_sb[:])
                    del strips[(h, s)]
                    if s == NSTRIP - 1:
                        nc.sync.dma_start(out=den[h], in_=den_sb[:])

            load_head(0)
            pending = None
            for job in jobs:
                if job["h"] not in heads:
                    load_head(job["h"])
                pt = emit_front(job)
                if pending is not None:
                    emit_back(*pending)
                pending = (job, pt)
            emit_back(*pending)
    nc.compile()
    return nc


def get_nc():
    if "nc" not in _nc_cache:
        _nc_cache["nc"] = _build_nc()
    return _nc_cache["nc"]


def _build_const():
    dk = np.arange(128)[:, None]
    c = np.arange(128)[None, :]
    tri = np.where(dk <= c, 0.0, NEG).astype(np.float32)
    return np.tile(tri, (1, 4))


def make_in_maps(qkv):
    qkv = np.asarray(qkv, dtype=np.float32)
    qkv_b = qkv.astype(ml_dtypes.bfloat16)
    cst = _build_const()
    in_maps = []
    for core in range(NCORES):
        qkvT = np.empty((HPC, 128, 3 * S), ml_dtypes.bfloat16)
        for i in range(HPC):
            bh = core * HPC + i
            b, h = bh // H, bh % H
            qkvT[i, :, 0:S] = qkv_b[b, :, 0, h, :].T
            qkvT[i, :, S : 2 * S] = qkv_b[b, :, 1, h, :].T
            qkvT[i, :, 2 * S : 3 * S] = (
                qkv_b[b, :, 2, h, :]
                .reshape(S // KB, KB, D)
                .transpose(1, 0, 2)
                .reshape(KB, S)
            )
        in_maps.append({"qkvT": qkvT, "cst": cst})
    return in_maps


def assemble_out(results):
    out = np.empty((B, S, H, D), np.float32)
    for core in range(NCORES):
        oTc = results[core]["oT"]  # [HPC, 128, S]
        dnc = results[core]["den"]  # [HPC, 1, S]
        for i in range(HPC):
            bh = core * HPC + i
            b, h = bh // H, bh % H
            out[b, :, h, :] = oTc[i].T / dnc[i][0][:, None]
    return out


def kernel(qkv):
    from concourse.bass_utils import run_bass_kernel_spmd

    in_maps = make_in_maps(qkv)
    nc = get_nc()
    res = run_bass_kernel_spmd(nc, in_maps, list(range(NCORES)))
    return assemble_out(res.results)


# revision 7
# speedup vs baseline: 1.6862x; 1.1660x over previous
"""Causal multi-head attention (QKV-packed) on 8 Trainium2 NeuronCores.

Sharding: pure head-parallel. B*H = 32 (batch, head) pairs -> 4 per core,
zero inter-core communication. Per head, flash-style causal attention is
computed entirely in the "transposed" orientation so no on-device
transposes are needed:

  - Host pre-lays-out Q^T, K^T as bf16 [D=128, S] (D on partitions) and V
    as k-blocks [128, D]; scores are computed transposed S_T[k, q] =
    (K^T_j).T @ Q^T into fp32 PSUM "windows" of up to 1024 columns
    spanning 2 PSUM banks, so one ACT instruction exps a whole window
    (halves the ScalarE per-instruction overhead vs per-block exp).
  - The diagonal quad of each q-strip is split into 4 triangular 128-col
    segments (which share one contiguous [128,512] window region -> ONE
    DVE mask add of a precomputed TRI4 mask and ONE merged den matmul)
    plus 3 clean segments.
  - P_T = exp(scale * S_T + mask) lands in SBUF as bf16; O^T[d, q] +=
    V_j.T @ P_T and den[q] += ones.T @ P_T accumulate in PSUM per strip.
  - The PE instruction stream is software-pipelined one window deep:
    window i's PV/den matmuls (which wait on exp_i) are emitted AFTER
    window i+1's score matmuls, so the strict-FIFO PE queue always has
    independent work while ScalarE runs exp and never goes idle (idle
    gaps > ~3.4us re-throttle the PE clock to 1.2 GHz via HAM).
  - No normalization on device: O^T and den are DMA'd out and the host
    computes O^T / den (the on-device reciprocal was ~50us of DVE time).
  - All matmuls are bf16 (full-rate, FastWeightLoad on 128-col weights).
"""

import sys

if "/opt/trn_rl_repo" not in sys.path:
    sys.path.insert(0, "/opt/trn_rl_repo")

import numpy as np
import ml_dtypes

B, S, H, D = 2, 2048, 16, 128
NCORES = 8
HPC = (B * H) // NCORES  # heads per core = 4
QS = 512   # q-strip width (one PSUM bank of fp32 for O^T)
KB = 128   # k-block (partition dim)
WIN = 1024  # exp window columns (2 PSUM banks fp32)
NEG = -1.0e30
SCALE = 1.0 / float(np.sqrt(D))
NSTRIP = S // QS  # 4

_nc_cache = {}


def _strip_windows(s):
    """Windows for q-strip s. Each window is
    (score_segs, den_segs, used_cols, tri_cols) with score segments
    (j, qlo, qlen, woff): block j contributes strip-local q columns
    [qlo, qlo+qlen) placed at window columns [woff, woff+qlen). tri_cols
    is the width of the leading region needing the TRI4 causal mask add.
    den_segs are (qlo, qlen, woff) only; the 4 tri segments merge into
    one den matmul since woff == qlo throughout [0, 512)."""
    wins = []
    full = list(range(4 * s))
    for p in range(0, len(full), 2):
        segs = [(full[p], 0, QS, 0)]
        used = QS
        if p + 1 < len(full):
            segs.append((full[p + 1], 0, QS, QS))
            used = 2 * QS
        dsegs = [(qlo, qlen, woff) for _, qlo, qlen, woff in segs]
        wins.append((segs, dsegs, used, 0))
    J = 4 * s
    # diagonal window A: four 128-col triangular segments packed at window
    # cols [0,512) with woff == qlo + clean parts of t=0 and t=2; same-lhsT
    # segments adjacent in emission order.
    segs_a = [
        (J + 0, 0, 128, 0),
        (J + 0, 128, 384, 512),
        (J + 1, 128, 128, 128),
        (J + 2, 256, 128, 256),
        (J + 2, 384, 128, 896),
        (J + 3, 384, 128, 384),
    ]
    dsegs_a = [(0, 512, 0), (128, 384, 512), (384, 128, 896)]
    wins.append((segs_a, dsegs_a, 1024, 512))
    # diagonal window B: clean part of t=1
    wins.append(([(J + 1, 256, 256, 0)], [(256, 256, 0)], 256, 0))
    return wins


def _build_nc():
    import concourse.bass as bass  # noqa: F401
    import concourse.mybir as mybir
    from concourse import bacc
    from concourse.tile import TileContext

    f32 = mybir.dt.float32
    bf16 = mybir.dt.bfloat16
    Exp = mybir.ActivationFunctionType.Exp

    nc = bacc.Bacc()
    # Packed input per head [128, 3*S] bf16:
    # cols [0,S) = Q^T, [S,2S) = K^T, [2S,3S) = V swizzled so column
    # block j holds the V k-block [128, D] (v[p, j*KB+d] = V[j*KB+p, d]).
    qkvT = nc.declare_dram_parameter("qkvT", [HPC, 128, 3 * S], bf16, isOutput=False)
    cst = nc.declare_dram_parameter("cst", [128, 512], f32, isOutput=False)
    oT = nc.declare_dram_parameter("oT", [HPC, 128, S], f32, isOutput=True)
    den = nc.declare_dram_parameter("den", [HPC, 1, S], f32, isOutput=True)

    # Flat job list: one entry per exp-window, in execution order.
    jobs = []
    for h in range(HPC):
        for s in range(NSTRIP):
            wins = _strip_windows(s)
            for wi, (segs, dsegs, used, tri_cols) in enumerate(wins):
                jobs.append(
                    dict(
                        h=h, s=s, segs=segs, dsegs=dsegs, used=used,
                        tri=tri_cols, first=(wi == 0), last=(wi == len(wins) - 1),
                    )
                )

    with TileContext(nc) as tc:
        with (
            nc.allow_low_precision(reason="bf16 throughout; tol is 2e-2"),
            tc.tile_pool(name="cpool", bufs=1) as cpool,
            tc.tile_pool(name="qkpool", bufs=2) as qkpool,
            tc.tile_pool(name="ptpool", bufs=4) as ptpool,
            tc.tile_pool(name="obpool", bufs=2) as obpool,
            tc.tile_pool(name="dnpool", bufs=2) as dnpool,
            tc.tile_pool(name="psw", bufs=2, space="PSUM") as psw,
            tc.tile_pool(name="pso", bufs=2, space="PSUM") as pso,
            tc.tile_pool(name="psd", bufs=2, space="PSUM") as psd,
        ):
            tri4 = cpool.tile([128, 512], f32)
            nc.sync.dma_start(out=tri4[:], in_=cst[:])
            ones_col = cpool.tile([128, 1], bf16)
            nc.vector.memset(ones_col[:], 1.0)

            heads = {}   # h -> (qt_sb, kt_sb, v_sb, den_sb)
            strips = {}  # live strip state: (h, s) -> (o_ps, den_ps)

            def load_head(h):
                qkv_sb = qkpool.tile([128, 3 * S], bf16, tag="qkv_sb", name="qkv_sb")
                if h == 0:
                    # split the first head's load so the first matmuls can
                    # start early: K^T blocks 0-7, Q^T strips 0-1, V 0-7
                    # (1024-col = 2KB/partition chunks for DMA efficiency)
                    for c0, c1 in (
                        (S, S + 1024),
                        (0, 1024),
                        (2 * S, 2 * S + 1024),
                        (S + 1024, 2 * S),
                        (1024, S),
                        (2 * S + 1024, 3 * S),
                    ):
                        nc.sync.dma_start(out=qkv_sb[:, c0:c1], in_=qkvT[h][:, c0:c1])
                else:
                    nc.sync.dma_start(out=qkv_sb[:], in_=qkvT[h])
                den_sb = dnpool.tile([1, S], f32, tag="den_sb", name="den_sb")
                heads[h] = (
                    qkv_sb[:, 0:S],
                    qkv_sb[:, S : 2 * S],
                    qkv_sb[:, 2 * S : 3 * S],
                    den_sb,
                )

            def emit_front(job):
                """Score matmuls + mask + exp for one window; returns the
                pt tile for the back half."""
                h, s = job["h"], job["s"]
                qt_sb, kt_sb, _, _ = heads[h]
                if job["first"]:
                    strips[(h, s)] = (
                        pso.tile([128, QS], f32, tag="o_ps", name="o_ps"),
                        psd.tile([1, QS], f32, tag="den_ps", name="den_ps"),
                    )
                w = psw.tile([128, WIN], f32, tag="w")
                for j, qlo, qlen, woff in job["segs"]:
                    nc.tensor.matmul(
                        w[:, woff : woff + qlen],
                        lhsT=kt_sb[:, KB * j : KB * (j + 1)],
                        rhs=qt_sb[:, QS * s + qlo : QS * s + qlo + qlen],
                        start=True,
                        stop=True,
                    )
                if job["tri"]:
                    nc.vector.tensor_add(
                        w[:, 0 : job["tri"]], w[:, 0 : job["tri"]],
                        tri4[:, 0 : job["tri"]],
                    )
                pt = ptpool.tile([128, WIN], bf16, tag="pt")
                nc.scalar.activation(
                    pt[:, 0 : job["used"]], w[:, 0 : job["used"]], Exp, scale=SCALE
                )
                return pt

            def emit_back(job, pt):
                """PV + den matmuls (waiting on exp) and, for the last
                window of a strip, the strip epilogue."""
                h, s = job["h"], job["s"]
                _, _, v_sb, den_sb = heads[h]
                o_ps, den_ps = strips[(h, s)]
                for j, qlo, qlen, woff in job["segs"]:
                    nc.tensor.matmul(
                        o_ps[:, qlo : qlo + qlen],
                        lhsT=v_sb[:, KB * j : KB * (j + 1)],
                        rhs=pt[:, woff : woff + qlen],
                        start=job["first"] and (j, qlo) == job["segs"][0][:2],
                        stop=job["last"] and (j, qlo) == job["segs"][-1][:2],
                    )
                for di, (qlo, qlen, woff) in enumerate(job["dsegs"]):
                    nc.tensor.matmul(
                        den_ps[:, qlo : qlo + qlen],
                        lhsT=ones_col,
                        rhs=pt[:, woff : woff + qlen],
                        start=job["first"] and di == 0,
                        stop=job["last"] and di == len(job["dsegs"]) - 1,
                    )
                if job["last"]:
                    nc.vector.tensor_copy(den_sb[:, QS * s : QS * (s + 1)], den_ps[:])
                    o_sb = obpool.tile([128, QS], f32, tag="o_sb")
                    nc.vector.tensor_copy(o_sb[:], o_ps[:])
                    nc.sync.dma_start(out=oT[h][:, QS * s : QS * (s + 1)], in_=o_sb[:])
                    del strips[(h, s)]
                    if s == NSTRIP - 1:
                        nc.sync.dma_start(out=den[h], in_=den_sb[:])

            # Software pipeline, two windows deep: the PE queue always holds
            # two windows of independent score matmuls ahead of any
            # exp-dependent PV/den group. Next head's 1.5MB qkv DMA is
            # prefetched one strip into the current head.
            load_head(0)
            from collections import deque

            pending = deque()
            LAG = 2
            for job in jobs:
                if job["h"] + 1 < HPC and job["h"] + 1 not in heads and job[
                    "s"
                ] == 1 and job["first"]:
                    load_head(job["h"] + 1)
                pt = emit_front(job)
                pending.append((job, pt))
                if len(pending) > LAG:
                    emit_back(*pending.popleft())
            while pending:
                emit_back(*pending.popleft())
    nc.compile()
    return nc


def get_nc():
    if "nc" not in _nc_cache:
        _nc_cache["nc"] = _build_nc()
    return _nc_cache["nc"]


def _build_const():
    dk = np.arange(128)[:, None]
    c = np.arange(128)[None, :]
    tri = np.where(dk <= c, 0.0, NEG).astype(np.float32)
    return np.tile(tri, (1, 4))


def make_in_maps(qkv):
    qkv = np.asarray(qkv, dtype=np.float32)
    qkv_b = qkv.astype(ml_dtypes.bfloat16)
    cst = _build_const()
    in_maps = []
    for core in range(NCORES):
        qkvT = np.empty((HPC, 128, 3 * S), ml_dtypes.bfloat16)
        for i in range(HPC):
            bh = core * HPC + i
            b, h = bh // H, bh % H
            qkvT[i, :, 0:S] = qkv_b[b, :, 0, h, :].T
            qkvT[i, :, S : 2 * S] = qkv_b[b, :, 1, h, :].T
            qkvT[i, :, 2 * S : 3 * S] = (
                qkv_b[b, :, 2, h, :]
                .reshape(S // KB, KB, D)
                .transpose(1, 0, 2)
                .reshape(KB, S)
            )
        in_maps.append({"qkvT": qkvT, "cst": cst})
    return in_maps


def assemble_out(results):
    out = np.empty((B, S, H, D), np.float32)
    for core in range(NCORES):
        oTc = results[core]["oT"]  # [HPC, 128, S]
        dnc = results[core]["den"]  # [HPC, 1, S]
        for i in range(HPC):
            bh = core * HPC + i
            b, h = bh // H, bh % H
            out[b, :, h, :] = oTc[i].T / dnc[i][0][:, None]
    return out


def kernel(qkv):
    from concourse.bass_utils import run_bass_kernel_spmd

    in_maps = make_in_maps(qkv)
    nc = get_nc()
    res = run_bass_kernel_spmd(nc, in_maps, list(range(NCORES)))
    return assemble_out(res.results)


# revision 12
# speedup vs baseline: 1.7644x; 1.0464x over previous
"""Causal multi-head attention (QKV-packed) on 8 Trainium2 NeuronCores.

Sharding: pure head-parallel. B*H = 32 (batch, head) pairs -> 4 per core,
zero inter-core communication. Per head, flash-style causal attention is
computed entirely in the "transposed" orientation so no on-device
transposes are needed:

  - Host pre-lays-out Q^T, K^T as bf16 [D=128, S] (D on partitions) and V
    as k-blocks [128, D]; scores are computed transposed S_T[k, q] =
    (K^T_j).T @ Q^T into fp32 PSUM "windows" of up to 1024 columns
    spanning 2 PSUM banks, so one ACT instruction exps a whole window
    (halves the ScalarE per-instruction overhead vs per-block exp).
  - The diagonal quad of each q-strip is split into 4 triangular 128-col
    segments (which share one contiguous [128,512] window region -> ONE
    DVE mask add of a precomputed TRI4 mask and ONE merged den matmul)
    plus 3 clean segments.
  - P_T = exp(scale * S_T + mask) lands in SBUF as bf16; O^T[d, q] +=
    V_j.T @ P_T and den[q] += ones.T @ P_T accumulate in PSUM per strip.
  - The PE instruction stream is software-pipelined one window deep:
    window i's PV/den matmuls (which wait on exp_i) are emitted AFTER
    window i+1's score matmuls, so the strict-FIFO PE queue always has
    independent work while ScalarE runs exp and never goes idle (idle
    gaps > ~3.4us re-throttle the PE clock to 1.2 GHz via HAM).
  - No normalization on device: O^T and den are DMA'd out and the host
    computes O^T / den (the on-device reciprocal was ~50us of DVE time).
  - All matmuls are bf16 (full-rate, FastWeightLoad on 128-col weights).
"""

import sys

if "/opt/trn_rl_repo" not in sys.path:
    sys.path.insert(0, "/opt/trn_rl_repo")

import numpy as np
import ml_dtypes

B, S, H, D = 2, 2048, 16, 128
NCORES = 8
HPC = (B * H) // NCORES  # heads per core = 4
QS = 512   # q-strip width (one PSUM bank of fp32 for O^T)
KB = 128   # k-block (partition dim)
WIN = 1024  # exp window columns (2 PSUM banks fp32)
NEG = -1.0e30
SCALE = 1.0 / float(np.sqrt(D))
NSTRIP = S // QS  # 4

_nc_cache = {}


def _strip_windows(s):
    """Windows for q-strip s. Each window is
    (score_segs, den_segs, used_cols, tri_cols) with score segments
    (j, qlo, qlen, woff): block j contributes strip-local q columns
    [qlo, qlo+qlen) placed at window columns [woff, woff+qlen). tri_cols
    is the width of the leading region needing the TRI4 causal mask add.
    den_segs are (qlo, qlen, woff) only; the 4 tri segments merge into
    one den matmul since woff == qlo throughout [0, 512)."""
    wins = []
    full = list(range(4 * s))
    for p in range(0, len(full), 2):
        segs = [(full[p], 0, QS, 0)]
        used = QS
        if p + 1 < len(full):
            segs.append((full[p + 1], 0, QS, QS))
            used = 2 * QS
        dsegs = [(qlo, qlen, woff) for _, qlo, qlen, woff in segs]
        wins.append((segs, dsegs, used, 0))
    J = 4 * s
    # diagonal window A: four 128-col triangular segments packed at window
    # cols [0,512) with woff == qlo + clean parts of t=0 and t=2; same-lhsT
    # segments adjacent in emission order.
    segs_a = [
        (J + 0, 0, 128, 0),
        (J + 0, 128, 384, 512),
        (J + 1, 128, 128, 128),
        (J + 2, 256, 128, 256),
        (J + 2, 384, 128, 896),
        (J + 3, 384, 128, 384),
    ]
    dsegs_a = [(0, 512, 0), (128, 384, 512), (384, 128, 896)]
    wins.append((segs_a, dsegs_a, 1024, 512))
    # diagonal window B: clean part of t=1
    wins.append(([(J + 1, 256, 256, 0)], [(256, 256, 0)], 256, 0))
    return wins


def _build_nc():
    import concourse.bass as bass  # noqa: F401
    import concourse.mybir as mybir
    from concourse import bacc
    from concourse.tile import TileContext

    f32 = mybir.dt.float32
    bf16 = mybir.dt.bfloat16
    Exp = mybir.ActivationFunctionType.Exp

    nc = bacc.Bacc()
    # Packed input per head [128, 3*S] bf16:
    # cols [0,S) = Q^T, [S,2S) = K^T, [2S,3S) = V swizzled so column
    # block j holds the V k-block [128, D] (v[p, j*KB+d] = V[j*KB+p, d]).
    qkvT = nc.declare_dram_parameter("qkvT", [HPC, 128, 3 * S], bf16, isOutput=False)
    cst = nc.declare_dram_parameter("cst", [128, 512], bf16, isOutput=False)
    oT = nc.declare_dram_parameter("oT", [HPC, 128, S], f32, isOutput=True)
    den = nc.declare_dram_parameter("den", [HPC, 1, S], f32, isOutput=True)

    # Flat job list: one entry per exp-window, in execution order.
    jobs = []
    for h in range(HPC):
        for s in range(NSTRIP):
            wins = _strip_windows(s)
            for wi, (segs, dsegs, used, tri_cols) in enumerate(wins):
                jobs.append(
                    dict(
                        h=h, s=s, segs=segs, dsegs=dsegs, used=used,
                        tri=tri_cols, first=(wi == 0), last=(wi == len(wins) - 1),
                    )
                )

    with TileContext(nc) as tc:
        with (
            nc.allow_low_precision(reason="bf16 throughout; tol is 2e-2"),
            tc.tile_pool(name="cpool", bufs=1) as cpool,
            tc.tile_pool(name="qkpool", bufs=2) as qkpool,
            tc.tile_pool(name="ptpool", bufs=4) as ptpool,
            tc.tile_pool(name="obpool", bufs=2) as obpool,
            tc.tile_pool(name="dnpool", bufs=2) as dnpool,
            tc.tile_pool(name="psw", bufs=2, space="PSUM") as psw,
            tc.tile_pool(name="pso", bufs=2, space="PSUM") as pso,
            tc.tile_pool(name="psd", bufs=2, space="PSUM") as psd,
        ):
            tri01 = cpool.tile([128, 512], bf16)
            nc.sync.dma_start(out=tri01[:], in_=cst[:])
            ones_col = cpool.tile([128, 1], bf16)
            nc.vector.memset(ones_col[:], 1.0)

            heads = {}   # h -> (qt_sb, kt_sb, v_sb, den_sb)
            strips = {}  # live strip state: (h, s) -> (o_ps, den_ps)

            def load_head(h):
                qkv_sb = qkpool.tile([128, 3 * S], bf16, tag="qkv_sb", name="qkv_sb")
                if h == 0:
                    # split the first head's load across BOTH hardware DGE
                    # queues (sync + scalar) so the first window's K/Q land
                    # ~4us earlier than a single serial queue would deliver
                    for eng, c0, c1 in (
                        ("a", S, S + 512),          # K^T blocks 0-3
                        ("a", 0, 512),              # Q^T strip 0
                        ("s", S + 512, S + 1024),   # K^T blocks 4-7
                        ("a", 2 * S, 2 * S + 512),  # V blocks 0-3
                        ("s", 512, 1024),           # Q^T strip 1
                        ("a", 2 * S + 512, 2 * S + 1024),
                        ("s", S + 1024, 2 * S),
                        ("a", 1024, S),
                        ("s", 2 * S + 1024, 3 * S),
                    ):
                        e = nc.scalar if eng == "a" else nc.sync
                        e.dma_start(out=qkv_sb[:, c0:c1], in_=qkvT[h][:, c0:c1])
                else:
                    nc.sync.dma_start(out=qkv_sb[:], in_=qkvT[h])
                den_sb = dnpool.tile([1, S], f32, tag="den_sb", name="den_sb")
                heads[h] = (
                    qkv_sb[:, 0:S],
                    qkv_sb[:, S : 2 * S],
                    qkv_sb[:, 2 * S : 3 * S],
                    den_sb,
                )

            def emit_front(job):
                """Score matmuls + mask + exp for one window; returns the
                pt tile for the back half."""
                h, s = job["h"], job["s"]
                qt_sb, kt_sb, _, _ = heads[h]
                if job["first"]:
                    strips[(h, s)] = (
                        pso.tile([128, QS], f32, tag="o_ps", name="o_ps"),
                        psd.tile([1, QS], f32, tag="den_ps", name="den_ps"),
                    )
                w = psw.tile([128, WIN], f32, tag="w")
                for j, qlo, qlen, woff in job["segs"]:
                    nc.tensor.matmul(
                        w[:, woff : woff + qlen],
                        lhsT=kt_sb[:, KB * j : KB * (j + 1)],
                        rhs=qt_sb[:, QS * s + qlo : QS * s + qlo + qlen],
                        start=True,
                        stop=True,
                    )
                pt = ptpool.tile([128, WIN], bf16, tag="pt")
                nc.scalar.activation(
                    pt[:, 0 : job["used"]], w[:, 0 : job["used"]], Exp, scale=SCALE
                )
                if job["tri"]:
                    # multiplicative causal mask (0/1) on the exp'd weights;
                    # off ScalarE's critical path, bf16 2x rate on DVE
                    nc.vector.tensor_mul(
                        pt[:, 0 : job["tri"]], pt[:, 0 : job["tri"]],
                        tri01[:, 0 : job["tri"]],
                    )
                return pt

            def emit_back(job, pt):
                """PV + den matmuls (waiting on exp) and, for the last
                window of a strip, the strip epilogue."""
                h, s = job["h"], job["s"]
                _, _, v_sb, den_sb = heads[h]
                o_ps, den_ps = strips[(h, s)]
                for j, qlo, qlen, woff in job["segs"]:
                    nc.tensor.matmul(
                        o_ps[:, qlo : qlo + qlen],
                        lhsT=v_sb[:, KB * j : KB * (j + 1)],
                        rhs=pt[:, woff : woff + qlen],
                        start=job["first"] and (j, qlo) == job["segs"][0][:2],
                        stop=job["last"] and (j, qlo) == job["segs"][-1][:2],
                    )
                for di, (qlo, qlen, woff) in enumerate(job["dsegs"]):
                    nc.tensor.matmul(
                        den_ps[:, qlo : qlo + qlen],
                        lhsT=ones_col,
                        rhs=pt[:, woff : woff + qlen],
                        start=job["first"] and di == 0,
                        stop=job["last"] and di == len(job["dsegs"]) - 1,
                    )
                if job["last"]:
                    nc.vector.tensor_copy(den_sb[:, QS * s : QS * (s + 1)], den_ps[:])
                    o_sb = obpool.tile([128, QS], f32, tag="o_sb")
                    nc.vector.tensor_copy(o_sb[:], o_ps[:])
                    nc.sync.dma_start(out=oT[h][:, QS * s : QS * (s + 1)], in_=o_sb[:])
                    del strips[(h, s)]
                    if s == NSTRIP - 1:
                        nc.sync.dma_start(out=den[h], in_=den_sb[:])

            # Software pipeline, two windows deep: the PE queue always holds
            # two windows of independent score matmuls ahead of any
            # exp-dependent PV/den group. Next head's 1.5MB qkv DMA is
            # prefetched one strip into the current head.
            load_head(0)
            from collections import deque

            pending = deque()
            LAG = 2
            for job in jobs:
                if job["h"] + 1 < HPC and job["h"] + 1 not in heads and job[
                    "s"
                ] == 1 and job["first"]:
                    load_head(job["h"] + 1)
                pt = emit_front(job)
                pending.append((job, pt))
                if len(pending) > LAG:
                    emit_back(*pending.popleft())
            while pending:
                emit_back(*pending.popleft())
    nc.compile()
    return nc


def get_nc():
    if "nc" not in _nc_cache:
        _nc_cache["nc"] = _build_nc()
    return _nc_cache["nc"]


def _build_const():
    dk = np.arange(128)[:, None]
    c = np.arange(128)[None, :]
    tri = np.where(dk <= c, 1.0, 0.0).astype(ml_dtypes.bfloat16)
    return np.tile(tri, (1, 4))


def make_in_maps(qkv):
    qkv = np.asarray(qkv, dtype=np.float32)
    qkv_b = qkv.astype(ml_dtypes.bfloat16)
    cst = _build_const()
    in_maps = []
    for core in range(NCORES):
        qkvT = np.empty((HPC, 128, 3 * S), ml_dtypes.bfloat16)
        for i in range(HPC):
            bh = core * HPC + i
            b, h = bh // H, bh % H
            qkvT[i, :, 0:S] = qkv_b[b, :, 0, h, :].T
            qkvT[i, :, S : 2 * S] = qkv_b[b, :, 1, h, :].T
            qkvT[i, :, 2 * S : 3 * S] = (
                qkv_b[b, :, 2, h, :]
                .reshape(S // KB, KB, D)
                .transpose(1, 0, 2)
                .reshape(KB, S)
            )
        in_maps.append({"qkvT": qkvT, "cst": cst})
    return in_maps


def assemble_out(results):
    out = np.empty((B, S, H, D), np.float32)
    for core in range(NCORES):
        oTc = results[core]["oT"]  # [HPC, 128, S]
        dnc = results[core]["den"]  # [HPC, 1, S]
        for i in range(HPC):
            bh = core * HPC + i
            b, h = bh // H, bh % H
            out[b, :, h, :] = oTc[i].T / dnc[i][0][:, None]
    return out


def kernel(qkv):
    from concourse.bass_utils import run_bass_kernel_spmd

    in_maps = make_in_maps(qkv)
    nc = get_nc()
    res = run_bass_kernel_spmd(nc, in_maps, list(range(NCORES)))
    return assemble_out(res.results)


# revision 14
# speedup vs baseline: 1.9260x; 1.0916x over previous
"""Causal multi-head attention (QKV-packed) on 8 Trainium2 NeuronCores.

Sharding: pure head-parallel. B*H = 32 (batch, head) pairs -> 4 per core,
zero inter-core communication. Per head, flash-style causal attention is
computed entirely in the "transposed" orientation so no on-device
transposes are needed:

  - Host pre-lays-out Q^T, K^T as bf16 [D=128, S] (D on partitions) and V
    as k-blocks [128, D]; scores are computed transposed S_T[k, q] =
    (K^T_j).T @ Q^T into fp32 PSUM "windows" of up to 1024 columns
    spanning 2 PSUM banks, so one ACT instruction exps a whole window
    (halves the ScalarE per-instruction overhead vs per-block exp).
  - The diagonal quad of each q-strip is split into 4 triangular 128-col
    segments (which share one contiguous [128,512] window region -> ONE
    DVE mask add of a precomputed TRI4 mask and ONE merged den matmul)
    plus 3 clean segments.
  - P_T = exp(scale * S_T + mask) lands in SBUF as bf16; O^T[d, q] +=
    V_j.T @ P_T and den[q] += ones.T @ P_T accumulate in PSUM per strip.
  - The PE instruction stream is software-pipelined one window deep:
    window i's PV/den matmuls (which wait on exp_i) are emitted AFTER
    window i+1's score matmuls, so the strict-FIFO PE queue always has
    independent work while ScalarE runs exp and never goes idle (idle
    gaps > ~3.4us re-throttle the PE clock to 1.2 GHz via HAM).
  - No normalization on device: O^T and den are DMA'd out and the host
    computes O^T / den (the on-device reciprocal was ~50us of DVE time).
  - All matmuls are bf16 (full-rate, FastWeightLoad on 128-col weights).
"""

import sys

if "/opt/trn_rl_repo" not in sys.path:
    sys.path.insert(0, "/opt/trn_rl_repo")

import numpy as np
import ml_dtypes

B, S, H, D = 2, 2048, 16, 128
NCORES = 8
HPC = (B * H) // NCORES  # heads per core = 4
QS = 512   # q-strip width (one PSUM bank of fp32 for O^T)
KB = 128   # k-block (partition dim)
WIN = 1024  # exp window columns (2 PSUM banks fp32)
NEG = -1.0e30
SCALE = 1.0 / float(np.sqrt(D))
NSTRIP = S // QS  # 4

_nc_cache = {}


def _strip_windows(s):
    """Windows for q-strip s. Each window is
    (score_segs, den_segs, used_cols, tri_cols) with score segments
    (j, qlo, qlen, woff): block j contributes strip-local q columns
    [qlo, qlo+qlen) placed at window columns [woff, woff+qlen). tri_cols
    is the width of the leading region needing the TRI4 causal mask add.
    den_segs are (qlo, qlen, woff) only; the 4 tri segments merge into
    one den matmul since woff == qlo throughout [0, 512)."""
    wins = []
    full = list(range(4 * s))
    for p in range(0, len(full), 2):
        segs = [(full[p], 0, QS, 0)]
        used = QS
        if p + 1 < len(full):
            segs.append((full[p + 1], 0, QS, QS))
            used = 2 * QS
        dsegs = [(qlo, qlen, woff) for _, qlo, qlen, woff in segs]
        wins.append((segs, dsegs, used, 0))
    J = 4 * s
    # diagonal window A: four 128-col triangular segments packed at window
    # cols [0,512) with woff == qlo + clean parts of t=0 and t=2; same-lhsT
    # segments adjacent in emission order.
    segs_a = [
        (J + 0, 0, 128, 0),
        (J + 0, 128, 384, 512),
        (J + 1, 128, 128, 128),
        (J + 2, 256, 128, 256),
        (J + 2, 384, 128, 896),
        (J + 3, 384, 128, 384),
    ]
    dsegs_a = [(0, 512, 0), (128, 384, 512), (384, 128, 896)]
    wins.append((segs_a, dsegs_a, 1024, 512))
    # diagonal window B: clean part of t=1
    wins.append(([(J + 1, 256, 256, 0)], [(256, 256, 0)], 256, 0))
    return wins


def _build_nc():
    import concourse.bass as bass  # noqa: F401
    import concourse.mybir as mybir
    from concourse import bacc
    from concourse.tile import TileContext

    f32 = mybir.dt.float32
    bf16 = mybir.dt.bfloat16
    Exp = mybir.ActivationFunctionType.Exp

    nc = bacc.Bacc()
    # Packed input per head [128, 3*S] bf16:
    # cols [0,S) = Q^T, [S,2S) = K^T, [2S,3S) = V swizzled so column
    # block j holds the V k-block [128, D] (v[p, j*KB+d] = V[j*KB+p, d]).
    qkvT = nc.declare_dram_parameter("qkvT", [HPC, 128, 3 * S], bf16, isOutput=False)
    cst = nc.declare_dram_parameter("cst", [128, 512], bf16, isOutput=False)
    oT = nc.declare_dram_parameter("oT", [HPC, 128, S], f32, isOutput=True)
    den = nc.declare_dram_parameter("den", [HPC, 1, S], f32, isOutput=True)

    # Flat job list: one entry per exp-window, in execution order.
    jobs = []
    for h in range(HPC):
        for s in range(NSTRIP):
            wins = _strip_windows(s)
            for wi, (segs, dsegs, used, tri_cols) in enumerate(wins):
                jobs.append(
                    dict(
                        h=h, s=s, segs=segs, dsegs=dsegs, used=used,
                        tri=tri_cols, first=(wi == 0), last=(wi == len(wins) - 1),
                    )
                )

    with TileContext(nc) as tc:
        with (
            nc.allow_low_precision(reason="bf16 throughout; tol is 2e-2"),
            tc.tile_pool(name="cpool", bufs=1) as cpool,
            tc.tile_pool(name="qkpool", bufs=2) as qkpool,
            tc.tile_pool(name="ptpool", bufs=4) as ptpool,
            tc.tile_pool(name="obpool", bufs=2) as obpool,
            tc.tile_pool(name="dnpool", bufs=2) as dnpool,
            tc.tile_pool(name="papool", bufs=3) as papool,
            tc.tile_pool(name="psw", bufs=2, space="PSUM") as psw,
            tc.tile_pool(name="pso", bufs=2, space="PSUM") as pso,
            tc.tile_pool(name="psd", bufs=2, space="PSUM") as psd,
        ):
            tri01 = cpool.tile([128, 512], bf16)
            nc.sync.dma_start(out=tri01[:], in_=cst[:])
            ones_col = cpool.tile([128, 1], bf16)
            nc.vector.memset(ones_col[:], 1.0)

            heads = {}   # h -> (qt_sb, kt_sb, v_sb, den_sb)
            strips = {}  # live strip state: (h, s) -> (o_ps, den_ps)

            def load_head(h):
                qkv_sb = qkpool.tile([128, 3 * S], bf16, tag="qkv_sb", name="qkv_sb")
                if h == 0:
                    # split the first head's load across BOTH hardware DGE
                    # queues (sync + scalar) so the first window's K/Q land
                    # ~4us earlier than a single serial queue would deliver
                    for eng, c0, c1 in (
                        ("a", S, S + 512),          # K^T blocks 0-3
                        ("a", 0, 512),              # Q^T strip 0
                        ("s", S + 512, S + 1024),   # K^T blocks 4-7
                        ("a", 2 * S, 2 * S + 512),  # V blocks 0-3
                        ("s", 512, 1024),           # Q^T strip 1
                        ("a", 2 * S + 512, 2 * S + 1024),
                        ("s", S + 1024, 2 * S),
                        ("a", 1024, S),
                        ("s", 2 * S + 1024, 3 * S),
                    ):
                        e = nc.scalar if eng == "a" else nc.sync
                        e.dma_start(out=qkv_sb[:, c0:c1], in_=qkvT[h][:, c0:c1])
                else:
                    nc.sync.dma_start(out=qkv_sb[:], in_=qkvT[h])
                den_sb = dnpool.tile([1, S], f32, tag="den_sb", name="den_sb")
                heads[h] = (
                    qkv_sb[:, 0:S],
                    qkv_sb[:, S : 2 * S],
                    qkv_sb[:, 2 * S : 3 * S],
                    den_sb,
                )

            def emit_front(job):
                """Score matmuls + mask + exp for one window; returns the
                pt tile for the back half."""
                h, s = job["h"], job["s"]
                qt_sb, kt_sb, _, _ = heads[h]
                if job["first"]:
                    strips[(h, s)] = (
                        pso.tile([128, QS], f32, tag="o_ps", name="o_ps"),
                        psd.tile([1, QS], f32, tag="den_ps", name="den_ps"),
                    )
                w = psw.tile([128, WIN], f32, tag="w")
                for j, qlo, qlen, woff in job["segs"]:
                    nc.tensor.matmul(
                        w[:, woff : woff + qlen],
                        lhsT=kt_sb[:, KB * j : KB * (j + 1)],
                        rhs=qt_sb[:, QS * s + qlo : QS * s + qlo + qlen],
                        start=True,
                        stop=True,
                    )
                pt = ptpool.tile([128, WIN], bf16, tag="pt")
                nc.scalar.activation(
                    pt[:, 0 : job["used"]], w[:, 0 : job["used"]], Exp, scale=SCALE
                )
                if job["tri"]:
                    # multiplicative causal mask (0/1) on the exp'd weights;
                    # off ScalarE's critical path, bf16 2x rate on DVE
                    nc.vector.tensor_mul(
                        pt[:, 0 : job["tri"]], pt[:, 0 : job["tri"]],
                        tri01[:, 0 : job["tri"]],
                    )
                return pt

            def emit_back(job, pt):
                """PV + den matmuls (waiting on exp) and, for the last
                window of a strip, the strip epilogue."""
                h, s = job["h"], job["s"]
                _, _, v_sb, den_sb = heads[h]
                o_ps, den_ps = strips[(h, s)]
                for j, qlo, qlen, woff in job["segs"]:
                    nc.tensor.matmul(
                        o_ps[:, qlo : qlo + qlen],
                        lhsT=v_sb[:, KB * j : KB * (j + 1)],
                        rhs=pt[:, woff : woff + qlen],
                        start=job["first"] and (j, qlo) == job["segs"][0][:2],
                        stop=job["last"] and (j, qlo) == job["segs"][-1][:2],
                    )
                dsegs = job["dsegs"]
                if len(dsegs) == 2 and dsegs[0] == (0, QS, 0) and dsegs[1] == (
                    0, QS, QS,
                ):
                    # full window: both slabs cover the same q range; pre-sum
                    # them on the (idle) DVE so den needs ONE PE matmul
                    pt01 = papool.tile([128, QS], bf16, tag="pt01", name="pt01")
                    nc.vector.tensor_add(pt01[:], pt[:, 0:QS], pt[:, QS : 2 * QS])
                    dsrc = [(0, QS, 0, pt01)]
                else:
                    dsrc = [(qlo, qlen, woff, pt) for qlo, qlen, woff in dsegs]
                for di, (qlo, qlen, woff, src) in enumerate(dsrc):
                    nc.tensor.matmul(
                        den_ps[:, qlo : qlo + qlen],
                        lhsT=ones_col,
                        rhs=src[:, woff : woff + qlen],
                        start=job["first"] and di == 0,
                        stop=job["last"] and di == len(dsrc) - 1,
                    )
                if job["last"]:
                    nc.vector.tensor_copy(den_sb[:, QS * s : QS * (s + 1)], den_ps[:])
                    o_sb = obpool.tile([128, QS], f32, tag="o_sb")
                    nc.vector.tensor_copy(o_sb[:], o_ps[:])
                    nc.sync.dma_start(out=oT[h][:, QS * s : QS * (s + 1)], in_=o_sb[:])
                    del strips[(h, s)]
                    if s == NSTRIP - 1:
                        nc.sync.dma_start(out=den[h], in_=den_sb[:])

            # Software pipeline, two windows deep: the PE queue always holds
            # two windows of independent score matmuls ahead of any
            # exp-dependent PV/den group. Next head's 1.5MB qkv DMA is
            # prefetched one strip into the current head.
            load_head(0)
            from collections import deque

            pending = deque()
            LAG = 2
            for job in jobs:
                if job["h"] + 1 < HPC and job["h"] + 1 not in heads and job[
                    "s"
                ] == 1 and job["first"]:
                    load_head(job["h"] + 1)
                pt = emit_front(job)
                pending.append((job, pt))
                if len(pending) > LAG:
                    emit_back(*pending.popleft())
            while pending:
                emit_back(*pending.popleft())
    nc.compile()
    return nc


def get_nc():
    if "nc" not in _nc_cache:
        _nc_cache["nc"] = _build_nc()
    return _nc_cache["nc"]


def _build_const():
    dk = np.arange(128)[:, None]
    c = np.arange(128)[None, :]
    tri = np.where(dk <= c, 1.0, 0.0).astype(ml_dtypes.bfloat16)
    return np.tile(tri, (1, 4))


def make_in_maps(qkv):
    qkv = np.asarray(qkv, dtype=np.float32)
    qkv_b = qkv.astype(ml_dtypes.bfloat16)
    cst = _build_const()
    in_maps = []
    for core in range(NCORES):
        qkvT = np.empty((HPC, 128, 3 * S), ml_dtypes.bfloat16)
        for i in range(HPC):
            bh = core * HPC + i
            b, h = bh // H, bh % H
            qkvT[i, :, 0:S] = qkv_b[b, :, 0, h, :].T
            qkvT[i, :, S : 2 * S] = qkv_b[b, :, 1, h, :].T
            qkvT[i, :, 2 * S : 3 * S] = (
                qkv_b[b, :, 2, h, :]
                .reshape(S // KB, KB, D)
                .transpose(1, 0, 2)
                .reshape(KB, S)
            )
        in_maps.append({"qkvT": qkvT, "cst": cst})
    return in_maps


def assemble_out(results):
    out = np.empty((B, S, H, D), np.float32)
    for core in range(NCORES):
        oTc = results[core]["oT"]  # [HPC, 128, S]
        dnc = results[core]["den"]  # [HPC, 1, S]
        for i in range(HPC):
            bh = core * HPC + i
            b, h = bh // H, bh % H
            out[b, :, h, :] = oTc[i].T / dnc[i][0][:, None]
    return out


def kernel(qkv):
    from concourse.bass_utils import run_bass_kernel_spmd

    in_maps = make_in_maps(qkv)
    nc = get_nc()
    res = run_bass_kernel_spmd(nc, in_maps, list(range(NCORES)))
    return assemble_out(res.results)
